# revision 4
# baseline (speedup 1.0000x reference)
"""GaussianNB log-posterior kernel for 8 Trainium2 NeuronCores.

out[b, c] = log_pi[c] - 0.5 * sum_f(log2pi + log_var[c,f] + (x[b,f]-mu[c,f])^2 / var[c,f])

Strategy v2: data-parallel over batch (B=2048 -> 256 rows/core), mu/log_var
replicated; host casts x/mu/log_var to bf16 (halves HBM traffic; rel err
~3e-4 vs the 2e-2 gate).

Per core:
  - DMA-xbar transposed loads: xT/muT/lvT land f-major [128, 8k, 256] bf16
    directly from DRAM (no PE transposes at all).
  - Elementwise prep in f-major on DVE/ACT/GPSIMD: invT=exp(-lvT),
    wcT=muT*invT, wqT=-0.5*invT, x2T=xT^2, sT=lvT+muT*wcT.
  - PE: warmup matmuls during the DMA window (HAM clock ungate), a
    ones-stationary row-reduce of sT -> s_row[1,C], and the main GEMM
    outT[c,b] = sum_k wqT*x2T + wcT*xT with the per-class const folded in
    as a trailing rank-1 f32r matmul (const_row x ones_row).
Output per core is (C, 256) transposed slice; host reassembles.
"""
import sys

sys.path.insert(0, "/opt/trn_rl_repo")
import numpy as np
import ml_dtypes
import concourse.bacc as bacc
import concourse.mybir as mybir
from concourse.tile import TileContext
from concourse.bass_utils import run_bass_kernel_spmd

B, C, F = 2048, 256, 1024
NCORES = 8
BSH = B // NCORES  # 256
KT = F // 128      # 8 k-tiles
LOG_2PI = float(np.log(2.0 * np.pi))
F32 = mybir.dt.float32
F32R = mybir.dt.float32r
BF16 = mybir.dt.bfloat16
AX = mybir.AxisListType.X
OP = mybir.AluOpType
AF = mybir.ActivationFunctionType

N_WARMUP = 12

_CACHE = {}


def _build():
    nc = bacc.Bacc("TRN2", target_bir_lowering=False, debug=False, num_devices=NCORES)
    x_d = nc.dram_tensor("x", [BSH, F], BF16, kind="ExternalInput").ap()
    mu_d = nc.dram_tensor("mu", [C, F], BF16, kind="ExternalInput").ap()
    lv_d = nc.dram_tensor("lv", [C, F], BF16, kind="ExternalInput").ap()
    lp_d = nc.dram_tensor("lp", [1, C], F32, kind="ExternalInput").ap()
    out_d = nc.dram_tensor("out", [C, BSH], F32, kind="ExternalOutput").ap()

    with TileContext(nc) as tc:
        with (
            tc.tile_pool(name="sb", bufs=1) as sb,
            tc.tile_pool(name="pw", bufs=1, space="PSUM") as pw,
            tc.tile_pool(name="pr", bufs=1, space="PSUM") as pr,
            tc.tile_pool(name="po", bufs=2, space="PSUM") as po,
        ):
            # ---------- transposed DMA in (xbar) ----------
            lvT = sb.tile([128, KT, C], BF16, tag="lvT")
            muT = sb.tile([128, KT, C], BF16, tag="muT")
            xT = sb.tile([128, KT, BSH], BF16, tag="xT")
            lp = sb.tile([1, C], F32, tag="lp")
            nc.sync.dma_start_transpose(lvT[:], lv_d[:, :])
            nc.scalar.dma_start_transpose(muT[:], mu_d[:, :])
            nc.sync.dma_start_transpose(xT[:], x_d[:, :])
            nc.scalar.dma_start(out=lp[:], in_=lp_d[:, :])

            # ---------- constants ----------
            ones_big = sb.tile([128, 512], BF16, tag="ones")
            nc.gpsimd.memset(ones_big[:], 1.0)
            ones_col = sb.tile([128, 1], BF16, tag="onec")
            nc.gpsimd.memset(ones_col[:], 1.0)
            ones_row = sb.tile([1, BSH], F32, tag="oner")
            nc.gpsimd.memset(ones_row[:], 1.0)

            # ---------- PE warmup (HAM ungate) during DMA window ----------
            pg_w = pw.tile([128, 512], F32, tag="pgw")
            for _ in range(N_WARMUP):
                nc.tensor.matmul(
                    pg_w[:], ones_big[:, 0:128], ones_big[:], start=True, stop=True
                )

            # ---------- f-major elementwise prep ----------
            invT = sb.tile([128, KT, C], BF16, tag="invT")
            wcT = sb.tile([128, KT, C], BF16, tag="wcT")
            wqT = sb.tile([128, KT, C], BF16, tag="wqT")
            m2iT = sb.tile([128, KT, C], BF16, tag="m2iT")
            sT = sb.tile([128, KT, C], BF16, tag="sT")
            x2T = sb.tile([128, KT, BSH], BF16, tag="x2T")
            nc.vector.tensor_mul(x2T[:], xT[:], xT[:])
            nc.scalar.activation(invT[:], lvT[:], AF.Exp, scale=-1.0)
            nc.vector.tensor_mul(wcT[:], muT[:], invT[:])
            nc.gpsimd.tensor_scalar_mul(wqT[:], invT[:], -0.5)
            nc.gpsimd.tensor_mul(m2iT[:], muT[:], wcT[:])
            nc.vector.tensor_add(sT[:], lvT[:], m2iT[:])

            # ---------- row-reduce sum_f (lv + mu^2*inv) -> [1, C] ----------
            ps_row = pr.tile([1, C], F32, tag="psr")
            for k in range(KT):
                nc.tensor.matmul(
                    ps_row[:], ones_col[:], sT[:, k, :],
                    start=(k == 0), stop=(k == KT - 1),
                )
            s_row = sb.tile([1, C], F32, tag="srow")
            nc.vector.tensor_copy(s_row[:], ps_row[:])
            # const_row = lp - 0.5*F*log2pi - 0.5*s  (f32r for the rank-1 MM)
            t_row = sb.tile([1, C], F32, tag="trow")
            nc.vector.tensor_scalar(
                t_row[:], s_row[:], -0.5, -0.5 * F * LOG_2PI, OP.mult, OP.add
            )
            const_row = sb.tile([1, C], F32R, tag="crow")
            nc.vector.tensor_add(const_row[:], t_row[:], lp[:])

            # ---------- GEMM + fused const + epilogue ----------
            for m in range(2):
                pg = po.tile([128, BSH], F32, tag=f"pg{m}")
                step = 0
                for T, A in ((wqT, x2T), (wcT, xT)):
                    for k in range(KT):
                        nc.tensor.matmul(
                            pg[:],
                            T[:, k, m * 128:(m + 1) * 128],
                            A[:, k, :],
                            start=(step == 0),
                            stop=False,
                        )
                        step += 1
                nc.tensor.matmul(
                    pg[:],
                    const_row[:, m * 128:(m + 1) * 128],
                    ones_row[:].bitcast(F32R),
                    start=False,
                    stop=True,
                )
                out_sb = sb.tile([128, BSH], F32, tag=f"os{m}")
                if m == 0:
                    nc.scalar.copy(out=out_sb[:], in_=pg[:])
                    nc.sync.dma_start(out=out_d[0:128, :], in_=out_sb[:])
                else:
                    nc.vector.tensor_copy(out_sb[:], pg[:])
                    nc.scalar.dma_start(out=out_d[128:256, :], in_=out_sb[:])

    nc.compile()
    return nc


def get_nc():
    if "nc" not in _CACHE:
        _CACHE["nc"] = _build()
    return _CACHE["nc"]


def kernel(x, mu, log_var, log_pi):
    x = np.asarray(x, dtype=np.float32).astype(ml_dtypes.bfloat16)
    mu = np.ascontiguousarray(
        np.asarray(mu, dtype=np.float32).astype(ml_dtypes.bfloat16)
    )
    lv = np.ascontiguousarray(
        np.asarray(log_var, dtype=np.float32).astype(ml_dtypes.bfloat16)
    )
    lp = np.ascontiguousarray(
        np.asarray(log_pi, dtype=np.float32).reshape(1, C)
    )
    nc = get_nc()
    in_maps = [
        {"x": np.ascontiguousarray(x[c * BSH:(c + 1) * BSH]),
         "mu": mu, "lv": lv, "lp": lp}
        for c in range(NCORES)
    ]
    res = run_bass_kernel_spmd(nc, in_maps, list(range(NCORES)))
    out = np.empty((B, C), dtype=np.float32)
    for c in range(NCORES):
        out[c * BSH:(c + 1) * BSH, :] = res.results[c]["out"].T
    return out


# revision 7
# speedup vs baseline: 1.8062x; 1.8062x over previous
"""GaussianNB log-posterior kernel for 8 Trainium2 NeuronCores.

out[b, c] = log_pi[c] - 0.5 * sum_f(log2pi + log_var[c,f] + (x[b,f]-mu[c,f])^2 / var[c,f])

Strategy v3: data-parallel over batch (B=2048 -> 256 rows/core), mu/log_var
replicated; host casts x/mu/log_var to bf16 (halves HBM traffic; rel err
~3e-4 vs the 2e-2 gate).

Per core:
  - 3 batched bf16 DMA loads ([128, 2, 1024] views), fully concurrent.
  - PE transposes all tensors to f-major (bf16 1-pass + FWL identity) --
    they double as the HAM clock warmup.
  - Fused PSUM copybacks: invT=exp(-lvT) and x2T=xT^2 land via ACT
    activations straight from the transpose PSUM; muT/lvT/xT via DVE copy.
  - f-major prep on DVE: wcT=muT*invT, wqT=-0.5*invT, m2iT=muT*wcT,
    sT=lvT+m2iT.
  - PE: GEMM outT[c,b] = sum_k wcT*xT + wqT*x2T; per-class const computed
    as a ones-stationary row-reduce of sT then folded into the GEMM psum
    as a trailing rank-1 f32r matmul (const_row x ones_row).
Output per core is (C, 256) transposed slice; host reassembles.
"""
import sys

sys.path.insert(0, "/opt/trn_rl_repo")
import numpy as np
import ml_dtypes
import concourse.bacc as bacc
import concourse.mybir as mybir
from concourse.tile import TileContext
from concourse.bass_utils import run_bass_kernel_spmd
from concourse.masks import make_identity

B, C, F = 2048, 256, 1024
NCORES = 8
BSH = B // NCORES  # 256
KT = F // 128      # 8 k-tiles
LOG_2PI = float(np.log(2.0 * np.pi))
F32 = mybir.dt.float32
F32R = mybir.dt.float32r
BF16 = mybir.dt.bfloat16
AX = mybir.AxisListType.X
OP = mybir.AluOpType
AF = mybir.ActivationFunctionType

_CACHE = {}


def _build():
    nc = bacc.Bacc("TRN2", target_bir_lowering=False, debug=False, num_devices=NCORES)
    x_d = nc.dram_tensor("x", [BSH, F], BF16, kind="ExternalInput").ap()
    mu_d = nc.dram_tensor("mu", [C, F], BF16, kind="ExternalInput").ap()
    lv_d = nc.dram_tensor("lv", [C, F], BF16, kind="ExternalInput").ap()
    lp_d = nc.dram_tensor("lp", [1, C], F32, kind="ExternalInput").ap()
    out_d = nc.dram_tensor("out", [C, BSH], F32, kind="ExternalOutput").ap()

    with TileContext(nc) as tc:
        with (
            tc.tile_pool(name="sb", bufs=1) as sb,
            tc.tile_pool(name="tp", bufs=2, space="PSUM") as tp,
            tc.tile_pool(name="pr", bufs=1, space="PSUM") as pr,
            tc.tile_pool(name="po", bufs=2, space="PSUM") as po,
        ):
            # ---------- batched DMA in (natural layout, bf16) ----------
            lv_nat = sb.tile([128, 2, F], BF16, tag="lvn")
            mu_nat = sb.tile([128, 2, F], BF16, tag="mun")
            x_nat = sb.tile([128, 2, F], BF16, tag="xn")
            lp = sb.tile([1, C], F32, tag="lp")
            nc.sync.dma_start(out=lv_nat[:], in_=lv_d.rearrange("(m p) f -> p m f", p=128))
            nc.scalar.dma_start(out=mu_nat[:], in_=mu_d.rearrange("(m p) f -> p m f", p=128))
            nc.sync.dma_start(out=x_nat[:], in_=x_d.rearrange("(m p) f -> p m f", p=128))
            nc.scalar.dma_start(out=lp[:], in_=lp_d[:, :])

            # ---------- constants ----------
            ident = sb.tile([128, 128], F32, tag="id")
            make_identity(nc, ident[:])
            identb = sb.tile([128, 128], BF16, tag="idb")
            nc.vector.tensor_copy(identb[:], ident[:])
            ones_col = sb.tile([128, 1], BF16, tag="onec")
            nc.gpsimd.memset(ones_col[:], 1.0)
            ones_row = sb.tile([1, BSH], F32, tag="oner")
            nc.gpsimd.memset(ones_row[:], 1.0)

            # ---------- PE transposes (also HAM warmup) ----------
            # f-major tiles: [128f, KT, C] where free cols [c-tile0 | c-tile1]
            lvT = sb.tile([128, KT, C], BF16, tag="lvT")
            invT = sb.tile([128, KT, C], BF16, tag="invT")
            muT = sb.tile([128, KT, C], BF16, tag="muT")
            xT = sb.tile([128, KT, BSH], BF16, tag="xT")
            x2T = sb.tile([128, KT, BSH], BF16, tag="x2T")

            def transpose_m(nat, m, dsts):
                # transpose 8 k-tiles of nat[:, m, :] into one psum quad,
                # then fan out via engine-specific copybacks
                p = tp.tile([128, KT * 128], BF16, tag="tp")
                for k in range(KT):
                    nc.tensor.transpose(
                        p[:, k * 128:(k + 1) * 128],
                        nat[:, m, k * 128:(k + 1) * 128],
                        identb[:],
                    )
                pv = p[:].rearrange("p (k c) -> p k c", k=KT)
                for eng, dst in dsts:
                    dst_ap = dst[:, :, m * 128:(m + 1) * 128]
                    if eng == "exp":
                        nc.scalar.activation(dst_ap, pv, AF.Exp, scale=-1.0)
                    elif eng == "sq":
                        nc.scalar.activation(dst_ap, pv, AF.Square)
                    elif eng == "v":
                        nc.vector.tensor_copy(dst_ap, pv)
                    elif eng == "g":
                        nc.gpsimd.tensor_copy(dst_ap, pv)

            for m in range(2):
                transpose_m(lv_nat, m, [("exp", invT), ("v", lvT)])
            for m in range(2):
                transpose_m(mu_nat, m, [("v", muT)])
            for m in range(2):
                transpose_m(x_nat, m, [("v", xT), ("sq", x2T)])

            # ---------- f-major elementwise prep (DVE) ----------
            wcT = sb.tile([128, KT, C], BF16, tag="wcT")
            wqT = sb.tile([128, KT, C], BF16, tag="wqT")
            m2iT = sb.tile([128, KT, C], BF16, tag="m2iT")
            sT = sb.tile([128, KT, C], BF16, tag="sT")
            nc.vector.tensor_mul(wcT[:], muT[:], invT[:])
            nc.vector.tensor_scalar_mul(wqT[:], invT[:], -0.5)
            nc.vector.tensor_mul(m2iT[:], muT[:], wcT[:])
            nc.vector.tensor_add(sT[:], lvT[:], m2iT[:])

            # ---------- GEMM c-tile 0 (cross first: xT ready before x2T) ----
            pgs = []
            for m in range(2):
                pg = po.tile([128, BSH], F32, tag=f"pg{m}")
                pgs.append(pg)
                step = 0
                for T, A in ((wcT, xT), (wqT, x2T)):
                    for k in range(KT):
                        nc.tensor.matmul(
                            pg[:],
                            T[:, k, m * 128:(m + 1) * 128],
                            A[:, k, :],
                            start=(step == 0),
                            stop=False,
                        )
                        step += 1

            # ---------- row-reduce sum_f (lv + mu^2*inv) -> [1, C] ----------
            ps_row = pr.tile([1, C], F32, tag="psr")
            for k in range(KT):
                nc.tensor.matmul(
                    ps_row[:], ones_col[:], sT[:, k, :],
                    start=(k == 0), stop=(k == KT - 1),
                )
            s_row = sb.tile([1, C], F32, tag="srow")
            nc.vector.tensor_copy(s_row[:], ps_row[:])
            # const_row = lp - 0.5*F*log2pi - 0.5*s  (f32r for the rank-1 MM)
            t_row = sb.tile([1, C], F32, tag="trow")
            nc.vector.tensor_scalar(
                t_row[:], s_row[:], -0.5, -0.5 * F * LOG_2PI, OP.mult, OP.add
            )
            const_row = sb.tile([1, C], F32R, tag="crow")
            nc.vector.tensor_add(const_row[:], t_row[:], lp[:])

            # ---------- fused const + epilogue ----------
            for m in range(2):
                nc.tensor.matmul(
                    pgs[m][:],
                    const_row[:, m * 128:(m + 1) * 128],
                    ones_row[:].bitcast(F32R),
                    start=False,
                    stop=True,
                )
                out_sb = sb.tile([128, BSH], F32, tag=f"os{m}")
                if m == 0:
                    nc.scalar.copy(out=out_sb[:], in_=pgs[m][:])
                    nc.sync.dma_start(out=out_d[0:128, :], in_=out_sb[:])
                else:
                    nc.vector.tensor_copy(out_sb[:], pgs[m][:])
                    nc.scalar.dma_start(out=out_d[128:256, :], in_=out_sb[:])

    nc.compile()
    return nc


def get_nc():
    if "nc" not in _CACHE:
        _CACHE["nc"] = _build()
    return _CACHE["nc"]


def kernel(x, mu, log_var, log_pi):
    x = np.asarray(x, dtype=np.float32).astype(ml_dtypes.bfloat16)
    mu = np.ascontiguousarray(
        np.asarray(mu, dtype=np.float32).astype(ml_dtypes.bfloat16)
    )
    lv = np.ascontiguousarray(
        np.asarray(log_var, dtype=np.float32).astype(ml_dtypes.bfloat16)
    )
    lp = np.ascontiguousarray(
        np.asarray(log_pi, dtype=np.float32).reshape(1, C)
    )
    nc = get_nc()
    in_maps = [
        {"x": np.ascontiguousarray(x[c * BSH:(c + 1) * BSH]),
         "mu": mu, "lv": lv, "lp": lp}
        for c in range(NCORES)
    ]
    res = run_bass_kernel_spmd(nc, in_maps, list(range(NCORES)))
    out = np.empty((B, C), dtype=np.float32)
    for c in range(NCORES):
        out[c * BSH:(c + 1) * BSH, :] = res.results[c]["out"].T
    return out


# revision 11
# speedup vs baseline: 2.0243x; 1.1207x over previous
"""GaussianNB log-posterior kernel for 8 Trainium2 NeuronCores.

out[b, c] = log_pi[c] - 0.5 * sum_f(log2pi + log_var[c,f] + (x[b,f]-mu[c,f])^2 / var[c,f])

Strategy v4: data-parallel over batch (B=2048 -> 256 rows/core), mu/log_var
replicated; host casts x/mu/log_var to bf16 (halves HBM traffic; rel err
~3e-4 vs the 2e-2 gate).

Per core:
  - 3 batched bf16 DMA loads, spread over sync/scalar/gpsimd queues.
  - PE warmup matmuls during the DMA window (HAM clock ungate), then PE
    transposes of x/lv/mu to f-major (bf16 1-pass + FWL).
  - Fused copybacks: invT=exp(-lvT) and x2T=xT^2 via ACT straight from
    transpose PSUM; xT via DVE copy; mu/lv PSUM quads are read directly by
    the DVE prep ops (no muT/lvT SBUF copies):
      wcT=mu_ps*invT, wqT=-0.5*invT, m2iT=mu_ps*wcT, sT=lv_ps+m2iT.
  - PE: GEMM outT[c,b] = sum_k wcT*xT + wqT*x2T; per-class const via a
    ones-stationary row-reduce of sT, folded into the GEMM psum as a
    trailing rank-1 f32r matmul.
Output per core is (C, 256) transposed slice; host reassembles.
"""
import sys

sys.path.insert(0, "/opt/trn_rl_repo")
import numpy as np
import ml_dtypes
import concourse.bacc as bacc
import concourse.mybir as mybir
from concourse.tile import TileContext
from concourse.bass_utils import run_bass_kernel_spmd
from concourse.masks import make_identity

B, C, F = 2048, 256, 1024
NCORES = 8
BSH = B // NCORES  # 256
KT = F // 128      # 8 k-tiles
LOG_2PI = float(np.log(2.0 * np.pi))
F32 = mybir.dt.float32
F32R = mybir.dt.float32r
BF16 = mybir.dt.bfloat16
AX = mybir.AxisListType.X
OP = mybir.AluOpType
AF = mybir.ActivationFunctionType

N_WARMUP = 12

_CACHE = {}


def _build():
    nc = bacc.Bacc("TRN2", target_bir_lowering=False, debug=False, num_devices=NCORES)
    x_d = nc.dram_tensor("x", [BSH, F], BF16, kind="ExternalInput").ap()
    mu_d = nc.dram_tensor("mu", [C, F], BF16, kind="ExternalInput").ap()
    lv_d = nc.dram_tensor("lv", [C, F], BF16, kind="ExternalInput").ap()
    lp_d = nc.dram_tensor("lp", [1, C], F32, kind="ExternalInput").ap()
    out_d = nc.dram_tensor("out", [C, BSH], F32, kind="ExternalOutput").ap()

    with TileContext(nc) as tc:
        with (
            tc.tile_pool(name="sb", bufs=1) as sb,
            tc.tile_pool(name="tp", bufs=1, space="PSUM") as tp,
            tc.tile_pool(name="pr", bufs=1, space="PSUM") as pr,
            tc.tile_pool(name="po", bufs=1, space="PSUM") as po,
        ):
            # ---------- batched DMA in (natural layout, bf16) ----------
            lv_nat = sb.tile([128, 2, F], BF16, tag="lvn")
            mu_nat = sb.tile([128, 2, F], BF16, tag="mun")
            x_nat = sb.tile([128, 2, F], BF16, tag="xn")
            lp = sb.tile([1, C], F32, tag="lp")
            nc.sync.dma_start(out=x_nat[:], in_=x_d.rearrange("(m p) f -> p m f", p=128))
            nc.scalar.dma_start(out=lv_nat[:], in_=lv_d.rearrange("(m p) f -> p m f", p=128))
            nc.gpsimd.dma_start(out=mu_nat[:], in_=mu_d.rearrange("(m p) f -> p m f", p=128))
            nc.scalar.dma_start(out=lp[:], in_=lp_d[:, :])

            # ---------- constants ----------
            ident = sb.tile([128, 128], F32, tag="id")
            make_identity(nc, ident[:])
            identb = sb.tile([128, 128], BF16, tag="idb")
            nc.vector.tensor_copy(identb[:], ident[:])
            ones_big = sb.tile([128, 256], BF16, tag="oneb")
            nc.gpsimd.memset(ones_big[:], 1.0)
            ones_col = sb.tile([128, 1], BF16, tag="onec")
            nc.gpsimd.memset(ones_col[:], 1.0)
            ones_row = sb.tile([1, BSH], F32, tag="oner")
            nc.gpsimd.memset(ones_row[:], 1.0)

            # ---------- PE warmup (HAM ungate) during DMA window ----------
            # targets pg0; WAW-serialized before the real GEMM on the PE queue
            pg0 = po.tile([128, BSH], F32, tag="pg0")
            for _ in range(N_WARMUP):
                nc.tensor.matmul(
                    pg0[:], ones_big[:, 0:128], ones_big[:], start=True, stop=True
                )

            # ---------- PE transposes into PSUM quads ----------
            xT = sb.tile([128, KT, BSH], BF16, tag="xT")
            x2T = sb.tile([128, KT, BSH], BF16, tag="x2T")
            invT = sb.tile([128, KT, C], BF16, tag="invT")

            def transpose_m(nat, m, tag):
                p = tp.tile([128, KT * 128], BF16, tag=tag, name=tag)
                for k in range(KT):
                    nc.tensor.transpose(
                        p[:, k * 128:(k + 1) * 128],
                        nat[:, m, k * 128:(k + 1) * 128],
                        identb[:],
                    )
                return p[:].rearrange("p (k c) -> p k c", k=KT)

            lv_ps, mu_ps = [], []

            def x_round(m):
                # x quads share one psum tag (retire early); copy+square
                # fan-out immediately after
                ps = transpose_m(x_nat, m, "xp")
                sl = slice(m * 128, (m + 1) * 128)
                nc.vector.tensor_copy(xT[:, :, sl], ps)
                nc.scalar.activation(x2T[:, :, sl], ps, AF.Square)

            x_round(0)
            for m in range(2):
                lv_ps.append(transpose_m(lv_nat, m, f"lvp{m}"))
                sl = slice(m * 128, (m + 1) * 128)
                nc.scalar.activation(invT[:, :, sl], lv_ps[m], AF.Exp, scale=-1.0)
            x_round(1)
            for m in range(2):
                mu_ps.append(transpose_m(mu_nat, m, f"mup{m}"))

            # ---------- f-major elementwise prep (DVE, psum-direct) ----------
            wcT = sb.tile([128, KT, C], BF16, tag="wcT")
            wqT = sb.tile([128, KT, C], BF16, tag="wqT")
            m2iT = sb.tile([128, KT, C], BF16, tag="m2iT")
            sT = sb.tile([128, KT, C], BF16, tag="sT")
            for m in range(2):
                sl = slice(m * 128, (m + 1) * 128)
                nc.vector.tensor_mul(wcT[:, :, sl], mu_ps[m], invT[:, :, sl])
                nc.vector.tensor_scalar_mul(wqT[:, :, sl], invT[:, :, sl], -0.5)
                nc.vector.tensor_mul(m2iT[:, :, sl], mu_ps[m], wcT[:, :, sl])
                nc.vector.tensor_add(sT[:, :, sl], lv_ps[m], m2iT[:, :, sl])

            # ---------- GEMM (cross first: xT ready before x2T) ----------
            pg1 = po.tile([128, BSH], F32, tag="pg1")
            pgs = [pg0, pg1]
            for m in range(2):
                pg = pgs[m]
                step = 0
                for T, A in ((wcT, xT), (wqT, x2T)):
                    for k in range(KT):
                        nc.tensor.matmul(
                            pg[:],
                            T[:, k, m * 128:(m + 1) * 128],
                            A[:, k, :],
                            start=(step == 0),
                            stop=False,
                        )
                        step += 1

            # ---------- row-reduce sum_f (lv + mu^2*inv) -> [1, C] ----------
            ps_row = pr.tile([1, C], F32, tag="psr")
            for k in range(KT):
                nc.tensor.matmul(
                    ps_row[:], ones_col[:], sT[:, k, :],
                    start=(k == 0), stop=(k == KT - 1),
                )
            s_row = sb.tile([1, C], F32, tag="srow")
            nc.scalar.copy(out=s_row[:], in_=ps_row[:])
            # const_row = lp - 0.5*F*log2pi - 0.5*s  (f32r for the rank-1 MM)
            t_row = sb.tile([1, C], F32, tag="trow")
            nc.vector.tensor_scalar(
                t_row[:], s_row[:], -0.5, -0.5 * F * LOG_2PI, OP.mult, OP.add
            )
            const_row = sb.tile([1, C], F32R, tag="crow")
            nc.vector.tensor_add(const_row[:], t_row[:], lp[:])

            # ---------- fused const + epilogue ----------
            for m in range(2):
                nc.tensor.matmul(
                    pgs[m][:],
                    const_row[:, m * 128:(m + 1) * 128],
                    ones_row[:].bitcast(F32R),
                    start=False,
                    stop=True,
                )
                out_sb = sb.tile([128, BSH], F32, tag=f"os{m}")
                if m == 0:
                    nc.scalar.copy(out=out_sb[:], in_=pgs[m][:])
                    nc.sync.dma_start(out=out_d[0:128, :], in_=out_sb[:])
                else:
                    nc.vector.tensor_copy(out_sb[:], pgs[m][:])
                    nc.scalar.dma_start(out=out_d[128:256, :], in_=out_sb[:])

    nc.compile()
    return nc


def get_nc():
    if "nc" not in _CACHE:
        _CACHE["nc"] = _build()
    return _CACHE["nc"]


def kernel(x, mu, log_var, log_pi):
    x = np.asarray(x, dtype=np.float32).astype(ml_dtypes.bfloat16)
    mu = np.ascontiguousarray(
        np.asarray(mu, dtype=np.float32).astype(ml_dtypes.bfloat16)
    )
    lv = np.ascontiguousarray(
        np.asarray(log_var, dtype=np.float32).astype(ml_dtypes.bfloat16)
    )
    lp = np.ascontiguousarray(
        np.asarray(log_pi, dtype=np.float32).reshape(1, C)
    )
    nc = get_nc()
    in_maps = [
        {"x": np.ascontiguousarray(x[c * BSH:(c + 1) * BSH]),
         "mu": mu, "lv": lv, "lp": lp}
        for c in range(NCORES)
    ]
    res = run_bass_kernel_spmd(nc, in_maps, list(range(NCORES)))
    out = np.empty((B, C), dtype=np.float32)
    for c in range(NCORES):
        out[c * BSH:(c + 1) * BSH, :] = res.results[c]["out"].T
    return out


# revision 13
# speedup vs baseline: 2.1878x; 1.0808x over previous
"""GaussianNB log-posterior kernel for 8 Trainium2 NeuronCores.

out[b, c] = log_pi[c] - 0.5 * sum_f(log2pi + log_var[c,f] + (x[b,f]-mu[c,f])^2 / var[c,f])

Strategy v5: data-parallel over batch (B=2048 -> 256 rows/core), mu/log_var
replicated; host casts x/mu/log_var to bf16 and reads back bf16 output
(rel err ~2e-3 vs the 2e-2 gate).

Per core:
  - 6 per-m-tile bf16 DMA loads over sync/scalar queues (x first).
  - PE warmup matmuls (HAM ungate), then interleaved PE transposes
    (x.b0, lv.m0, mu.m0, x.b1, lv.m1, mu.m1) so the m0 weight chain
    starts earliest.
  - Fused copybacks: invT=exp(-lvT), x2T=xT^2 via ACT straight from
    transpose PSUM; xT via DVE copy; mu/lv PSUM read directly by DVE:
    wcT=mu_ps*invT, wqT=-0.5*invT, m2iT=mu_ps*wcT, sT=lv_ps+m2iT.
  - PE GEMM outT[c,b] = sum_k wcT*xT + wqT*x2T into pg_m[:, 0:256];
    per-m ones-stationary row-reduce of sT (+ a K=1 matmul folding
    -2*log_pi) lands in the same PSUM bank at pg_m[0:1, 256:384];
    const_row = -0.5*(row) - 0.5*F*log2pi via one fused tensor_scalar,
    then folded into the GEMM psum as a trailing rank-1 f32r matmul.
Output per core is (C, 256) bf16 transposed slice; host reassembles.
"""
import sys

sys.path.insert(0, "/opt/trn_rl_repo")
import numpy as np
import ml_dtypes
import concourse.bacc as bacc
import concourse.mybir as mybir
from concourse.tile import TileContext
from concourse.bass_utils import run_bass_kernel_spmd
from concourse.masks import make_identity

B, C, F = 2048, 256, 1024
NCORES = 8
BSH = B // NCORES  # 256
KT = F // 128      # 8 k-tiles
LOG_2PI = float(np.log(2.0 * np.pi))
F32 = mybir.dt.float32
F32R = mybir.dt.float32r
BF16 = mybir.dt.bfloat16
AX = mybir.AxisListType.X
OP = mybir.AluOpType
AF = mybir.ActivationFunctionType

N_WARMUP = 10

_CACHE = {}


def _build():
    nc = bacc.Bacc("TRN2", target_bir_lowering=False, debug=False, num_devices=NCORES)
    x_d = nc.dram_tensor("x", [BSH, F], BF16, kind="ExternalInput").ap()
    mu_d = nc.dram_tensor("mu", [C, F], BF16, kind="ExternalInput").ap()
    lv_d = nc.dram_tensor("lv", [C, F], BF16, kind="ExternalInput").ap()
    lp_d = nc.dram_tensor("lp", [1, C], F32, kind="ExternalInput").ap()
    out_d = nc.dram_tensor("out", [C, BSH], BF16, kind="ExternalOutput").ap()

    with TileContext(nc) as tc:
        with (
            tc.tile_pool(name="sb", bufs=1) as sb,
            tc.tile_pool(name="tp", bufs=1, space="PSUM") as tp,
            tc.tile_pool(name="po", bufs=1, space="PSUM") as po,
        ):
            # ---------- per-m-tile DMA in (bf16), x first ----------
            lv_nat = sb.tile([128, 2, F], BF16, tag="lvn")
            mu_nat = sb.tile([128, 2, F], BF16, tag="mun")
            x_nat = sb.tile([128, 2, F], BF16, tag="xn")
            lp = sb.tile([1, C], F32, tag="lp")
            xv = x_d.rearrange("(m p) f -> p m f", p=128)
            lvv = lv_d.rearrange("(m p) f -> p m f", p=128)
            muv = mu_d.rearrange("(m p) f -> p m f", p=128)
            nc.sync.dma_start(out=x_nat[:, 0], in_=xv[:, 0])
            nc.scalar.dma_start(out=x_nat[:, 1], in_=xv[:, 1])
            nc.sync.dma_start(out=lv_nat[:, 0], in_=lvv[:, 0])
            nc.scalar.dma_start(out=mu_nat[:, 0], in_=muv[:, 0])
            nc.sync.dma_start(out=lv_nat[:, 1], in_=lvv[:, 1])
            nc.scalar.dma_start(out=mu_nat[:, 1], in_=muv[:, 1])
            nc.scalar.dma_start(out=lp[:], in_=lp_d[:, :])

            # ---------- constants ----------
            ident = sb.tile([128, 128], F32, tag="id")
            make_identity(nc, ident[:])
            identb = sb.tile([128, 128], BF16, tag="idb")
            nc.vector.tensor_copy(identb[:], ident[:])
            ones_big = sb.tile([128, 256], BF16, tag="oneb")
            nc.gpsimd.memset(ones_big[:], 1.0)
            ones_col = sb.tile([128, 1], BF16, tag="onec")
            nc.gpsimd.memset(ones_col[:], 1.0)
            ones_row = sb.tile([1, BSH], F32, tag="oner")
            nc.gpsimd.memset(ones_row[:], 1.0)
            lpm2 = sb.tile([1, C], BF16, tag="lpm2")
            nc.vector.tensor_scalar_mul(lpm2[:], lp[:], -2.0)

            # ---------- PSUM layout ----------
            # pg_m bank: [:, 0:256] GEMM out (c-tile m); [0:1, 256:384] s-row
            pg0 = po.tile([128, 512], F32, tag="pg0")
            pg1 = po.tile([128, 512], F32, tag="pg1")
            pgs = [pg0, pg1]

            # ---------- PE warmup (HAM ungate) during DMA window ----------
            for _ in range(N_WARMUP):
                nc.tensor.matmul(
                    pg0[:, 0:BSH], ones_big[:, 0:128], ones_big[:],
                    start=True, stop=True,
                )

            # ---------- PE transposes into PSUM quads ----------
            xT = sb.tile([128, KT, BSH], BF16, tag="xT")
            x2T = sb.tile([128, KT, BSH], BF16, tag="x2T")
            invT = sb.tile([128, KT, C], BF16, tag="invT")

            def transpose_m(nat, m, tag):
                p = tp.tile([128, KT * 128], BF16, tag=tag, name=tag)
                for k in range(KT):
                    nc.tensor.transpose(
                        p[:, k * 128:(k + 1) * 128],
                        nat[:, m, k * 128:(k + 1) * 128],
                        identb[:],
                    )
                return p[:].rearrange("p (k c) -> p k c", k=KT)

            lv_ps, mu_ps = [None, None], [None, None]

            def x_round(b):
                ps = transpose_m(x_nat, b, f"xp{b}")
                sl = slice(b * 128, (b + 1) * 128)
                nc.vector.tensor_copy(xT[:, :, sl], ps)
                nc.scalar.activation(x2T[:, :, sl], ps, AF.Square)

            def w_round(m):
                lv_ps[m] = transpose_m(lv_nat, m, f"lvp{m}")
                sl = slice(m * 128, (m + 1) * 128)
                nc.scalar.activation(invT[:, :, sl], lv_ps[m], AF.Exp, scale=-1.0)
                mu_ps[m] = transpose_m(mu_nat, m, f"mup{m}")

            x_round(0)
            w_round(0)
            x_round(1)
            w_round(1)

            # ---------- f-major elementwise prep (DVE, psum-direct) ----------
            wcT = sb.tile([128, KT, C], BF16, tag="wcT")
            wqT = sb.tile([128, KT, C], BF16, tag="wqT")
            m2iT = sb.tile([128, KT, C], BF16, tag="m2iT")
            sT = sb.tile([128, KT, C], BF16, tag="sT")
            for m in range(2):
                sl = slice(m * 128, (m + 1) * 128)
                nc.vector.tensor_mul(wcT[:, :, sl], mu_ps[m], invT[:, :, sl])
                nc.vector.tensor_scalar_mul(wqT[:, :, sl], invT[:, :, sl], -0.5)
                nc.vector.tensor_mul(m2iT[:, :, sl], mu_ps[m], wcT[:, :, sl])
                nc.vector.tensor_add(sT[:, :, sl], lv_ps[m], m2iT[:, :, sl])

            # ---------- GEMM + per-m reduce + const + epilogue ----------
            const_row = sb.tile([1, C], F32R, tag="crow")

            def gemm_part(m, T, A):
                sl = slice(m * 128, (m + 1) * 128)
                for k in range(KT):
                    nc.tensor.matmul(
                        pgs[m][:, 0:BSH], T[:, k, sl], A[:, k, :],
                        start=(T is wcT and k == 0), stop=False,
                    )

            def reduce_m(m):
                sl = slice(m * 128, (m + 1) * 128)
                row = pgs[m][0:1, BSH:BSH + 128]
                # start=False everywhere: the GEMM's start=True already
                # cleared this bank (incl. the row region), so the first
                # write overwrites via has_written=0
                for k in range(KT):
                    nc.tensor.matmul(
                        row, ones_col[:], sT[:, k, sl],
                        start=False, stop=False, skip_group_check=True,
                    )
                nc.tensor.matmul(
                    row, ones_col[0:1, :], lpm2[:, sl],
                    start=False, stop=True, skip_group_check=True,
                )
                # const = -0.5*row - 0.5*F*log2pi  (one fused op from PSUM)
                nc.scalar.activation(
                    const_row[:, sl], row, AF.Copy,
                    bias=-0.5 * F * LOG_2PI, scale=-0.5,
                )

            def finish_m(m):
                sl = slice(m * 128, (m + 1) * 128)
                nc.tensor.matmul(
                    pgs[m][:, 0:BSH],
                    const_row[:, sl],
                    ones_row[:].bitcast(F32R),
                    start=False, stop=True, skip_group_check=True,
                )
                out_sb = sb.tile([128, BSH], BF16, tag=f"os{m}", name=f"os{m}")
                if m == 0:
                    nc.scalar.copy(out=out_sb[:], in_=pgs[m][:, 0:BSH])
                    nc.sync.dma_start(out=out_d[0:128, :], in_=out_sb[:])
                else:
                    nc.vector.tensor_copy(out_sb[:], pgs[m][:, 0:BSH])
                    nc.scalar.dma_start(out=out_d[128:256, :], in_=out_sb[:])

            gemm_part(0, wcT, xT)
            gemm_part(0, wqT, x2T)
            gemm_part(1, wcT, xT)
            reduce_m(0)
            finish_m(0)
            gemm_part(1, wqT, x2T)
            reduce_m(1)
            finish_m(1)

    nc.compile()
    return nc


def get_nc():
    if "nc" not in _CACHE:
        _CACHE["nc"] = _build()
    return _CACHE["nc"]


def kernel(x, mu, log_var, log_pi):
    x = np.asarray(x, dtype=np.float32).astype(ml_dtypes.bfloat16)
    mu = np.ascontiguousarray(
        np.asarray(mu, dtype=np.float32).astype(ml_dtypes.bfloat16)
    )
    lv = np.ascontiguousarray(
        np.asarray(log_var, dtype=np.float32).astype(ml_dtypes.bfloat16)
    )
    lp = np.ascontiguousarray(
        np.asarray(log_pi, dtype=np.float32).reshape(1, C)
    )
    nc = get_nc()
    in_maps = [
        {"x": np.ascontiguousarray(x[c * BSH:(c + 1) * BSH]),
         "mu": mu, "lv": lv, "lp": lp}
        for c in range(NCORES)
    ]
    res = run_bass_kernel_spmd(nc, in_maps, list(range(NCORES)))
    out = np.empty((B, C), dtype=np.float32)
    for c in range(NCORES):
        out[c * BSH:(c + 1) * BSH, :] = res.results[c]["out"].T.astype(np.float32)
    return out


# revision 14
# speedup vs baseline: 2.2299x; 1.0193x over previous
"""GaussianNB log-posterior kernel for 8 Trainium2 NeuronCores.

out[b, c] = log_pi[c] - 0.5 * sum_f(log2pi + log_var[c,f] + (x[b,f]-mu[c,f])^2 / var[c,f])

Strategy v5: data-parallel over batch (B=2048 -> 256 rows/core), mu/log_var
replicated; host casts x/mu/log_var to bf16 and reads back bf16 output
(rel err ~2e-3 vs the 2e-2 gate).

Per core:
  - 6 per-m-tile bf16 DMA loads over sync/scalar queues (x first).
  - PE warmup matmuls (HAM ungate), then interleaved PE transposes
    (x.b0, lv.m0, mu.m0, x.b1, lv.m1, mu.m1) so the m0 weight chain
    starts earliest.
  - Fused copybacks: invT=exp(-lvT), x2T=xT^2 via ACT straight from
    transpose PSUM; xT via DVE copy; mu/lv PSUM read directly by DVE:
    wcT=mu_ps*invT, wqT=-0.5*invT, m2iT=mu_ps*wcT, sT=lv_ps+m2iT.
  - PE GEMM outT[c,b] = sum_k wcT*xT + wqT*x2T into pg_m[:, 0:256];
    per-m ones-stationary row-reduce of sT (+ a K=1 matmul folding
    -2*log_pi) lands in the same PSUM bank at pg_m[0:1, 256:384];
    const_row = -0.5*(row) - 0.5*F*log2pi via one fused tensor_scalar,
    then folded into the GEMM psum as a trailing rank-1 f32r matmul.
Output per core is (C, 256) bf16 transposed slice; host reassembles.
"""
import sys

sys.path.insert(0, "/opt/trn_rl_repo")
import numpy as np
import ml_dtypes
import concourse.bacc as bacc
import concourse.mybir as mybir
from concourse.tile import TileContext
from concourse.bass_utils import run_bass_kernel_spmd
from concourse.masks import make_identity

B, C, F = 2048, 256, 1024
NCORES = 8
BSH = B // NCORES  # 256
KT = F // 128      # 8 k-tiles
LOG_2PI = float(np.log(2.0 * np.pi))
F32 = mybir.dt.float32
F32R = mybir.dt.float32r
BF16 = mybir.dt.bfloat16
AX = mybir.AxisListType.X
OP = mybir.AluOpType
AF = mybir.ActivationFunctionType

N_WARMUP = 10

_CACHE = {}


def _build():
    nc = bacc.Bacc("TRN2", target_bir_lowering=False, debug=False, num_devices=NCORES)
    x_d = nc.dram_tensor("x", [BSH, F], BF16, kind="ExternalInput").ap()
    mu_d = nc.dram_tensor("mu", [C, F], BF16, kind="ExternalInput").ap()
    lv_d = nc.dram_tensor("lv", [C, F], BF16, kind="ExternalInput").ap()
    lp_d = nc.dram_tensor("lp", [1, C], F32, kind="ExternalInput").ap()
    out_d = nc.dram_tensor("out", [C, BSH], BF16, kind="ExternalOutput").ap()

    with TileContext(nc) as tc:
        with (
            tc.tile_pool(name="sb", bufs=1) as sb,
            tc.tile_pool(name="tp", bufs=1, space="PSUM") as tp,
            tc.tile_pool(name="po", bufs=1, space="PSUM") as po,
        ):
            # ---------- per-m-tile DMA in (bf16), x first ----------
            lv_nat = sb.tile([128, 2, F], BF16, tag="lvn")
            mu_nat = sb.tile([128, 2, F], BF16, tag="mun")
            x_nat = sb.tile([128, 2, F], BF16, tag="xn")
            lp = sb.tile([1, C], F32, tag="lp")
            xv = x_d.rearrange("(m p) f -> p m f", p=128)
            lvv = lv_d.rearrange("(m p) f -> p m f", p=128)
            muv = mu_d.rearrange("(m p) f -> p m f", p=128)
            nc.sync.dma_start(out=x_nat[:, 0], in_=xv[:, 0])
            nc.scalar.dma_start(out=x_nat[:, 1], in_=xv[:, 1])
            nc.sync.dma_start(out=lv_nat[:, 0], in_=lvv[:, 0])
            nc.scalar.dma_start(out=mu_nat[:, 0], in_=muv[:, 0])
            nc.sync.dma_start(out=lv_nat[:, 1], in_=lvv[:, 1])
            nc.scalar.dma_start(out=mu_nat[:, 1], in_=muv[:, 1])
            nc.scalar.dma_start(out=lp[:], in_=lp_d[:, :])

            # ---------- constants ----------
            ident = sb.tile([128, 128], F32, tag="id")
            make_identity(nc, ident[:])
            identb = sb.tile([128, 128], BF16, tag="idb")
            nc.vector.tensor_copy(identb[:], ident[:])
            ones_big = sb.tile([128, 256], BF16, tag="oneb")
            nc.gpsimd.memset(ones_big[:], 1.0)
            ones_col = sb.tile([128, 1], BF16, tag="onec")
            nc.gpsimd.memset(ones_col[:], 1.0)
            ones_row = sb.tile([1, BSH], F32, tag="oner")
            nc.gpsimd.memset(ones_row[:], 1.0)
            lpm2 = sb.tile([1, C], BF16, tag="lpm2")
            nc.vector.tensor_scalar_mul(lpm2[:], lp[:], -2.0)

            # ---------- PSUM layout ----------
            # pg_m bank: [:, 0:256] GEMM out (c-tile m); [0:1, 256:384] s-row
            pg0 = po.tile([128, 512], F32, tag="pg0")
            pg1 = po.tile([128, 512], F32, tag="pg1")
            pgs = [pg0, pg1]

            # ---------- PE warmup (HAM ungate) during DMA window ----------
            for _ in range(N_WARMUP):
                nc.tensor.matmul(
                    pg0[:, 0:BSH], ones_big[:, 0:128], ones_big[:],
                    start=True, stop=True,
                )

            # ---------- PE transposes into PSUM quads ----------
            xT = sb.tile([128, KT, BSH], BF16, tag="xT")
            x2T = sb.tile([128, KT, BSH], BF16, tag="x2T")
            invT = sb.tile([128, KT, C], BF16, tag="invT")

            def transpose_m(nat, m, tag):
                p = tp.tile([128, KT * 128], BF16, tag=tag, name=tag)
                for k in range(KT):
                    nc.tensor.transpose(
                        p[:, k * 128:(k + 1) * 128],
                        nat[:, m, k * 128:(k + 1) * 128],
                        identb[:],
                    )
                return p[:].rearrange("p (k c) -> p k c", k=KT)

            lv_ps, mu_ps = [None, None], [None, None]

            def fillers(n):
                # keep the PE busy through DMA-wait gaps so HAM stays ungated
                for _ in range(n):
                    nc.tensor.matmul(
                        pg1[:, 0:BSH], ones_big[:, 0:128], ones_big[:],
                        start=True, stop=True,
                    )

            def x_round(b):
                ps = transpose_m(x_nat, b, f"xp{b}")
                sl = slice(b * 128, (b + 1) * 128)
                nc.vector.tensor_copy(xT[:, :, sl], ps)
                # x^2 via mixed SBUF*PSUM multiply (same values, distinct
                # ports -- dodges both the ACT queue and same-src slowness)
                nc.vector.tensor_mul(x2T[:, :, sl], xT[:, :, sl], ps)

            def w_round(m):
                lv_ps[m] = transpose_m(lv_nat, m, f"lvp{m}")
                sl = slice(m * 128, (m + 1) * 128)
                nc.scalar.activation(invT[:, :, sl], lv_ps[m], AF.Exp, scale=-1.0)
                mu_ps[m] = transpose_m(mu_nat, m, f"mup{m}")

            x_round(0)
            fillers(3)
            x_round(1)
            fillers(3)
            w_round(0)
            fillers(3)
            w_round(1)
            fillers(2)

            # ---------- f-major elementwise prep (DVE, psum-direct) ----------
            wcT = sb.tile([128, KT, C], BF16, tag="wcT")
            wqT = sb.tile([128, KT, C], BF16, tag="wqT")
            m2iT = sb.tile([128, KT, C], BF16, tag="m2iT")
            sT = sb.tile([128, KT, C], BF16, tag="sT")
            for m in range(2):
                sl = slice(m * 128, (m + 1) * 128)
                nc.vector.tensor_mul(wcT[:, :, sl], mu_ps[m], invT[:, :, sl])
                nc.vector.tensor_scalar_mul(wqT[:, :, sl], invT[:, :, sl], -0.5)
                nc.vector.tensor_mul(m2iT[:, :, sl], mu_ps[m], wcT[:, :, sl])
                nc.vector.tensor_add(sT[:, :, sl], lv_ps[m], m2iT[:, :, sl])

            # ---------- GEMM + per-m reduce + const + epilogue ----------
            const_row = sb.tile([1, C], F32R, tag="crow")

            def gemm_part(m, T, A):
                sl = slice(m * 128, (m + 1) * 128)
                for k in range(KT):
                    nc.tensor.matmul(
                        pgs[m][:, 0:BSH], T[:, k, sl], A[:, k, :],
                        start=(T is wcT and k == 0), stop=False,
                    )

            def reduce_m(m):
                sl = slice(m * 128, (m + 1) * 128)
                row = pgs[m][0:1, BSH:BSH + 128]
                # start=False everywhere: the GEMM's start=True already
                # cleared this bank (incl. the row region), so the first
                # write overwrites via has_written=0
                for k in range(KT):
                    nc.tensor.matmul(
                        row, ones_col[:], sT[:, k, sl],
                        start=False, stop=False, skip_group_check=True,
                    )
                nc.tensor.matmul(
                    row, ones_col[0:1, :], lpm2[:, sl],
                    start=False, stop=True, skip_group_check=True,
                )
                # const = -0.5*row - 0.5*F*log2pi  (one fused op from PSUM)
                nc.scalar.activation(
                    const_row[:, sl], row, AF.Copy,
                    bias=-0.5 * F * LOG_2PI, scale=-0.5,
                )

            def finish_m(m):
                sl = slice(m * 128, (m + 1) * 128)
                nc.tensor.matmul(
                    pgs[m][:, 0:BSH],
                    const_row[:, sl],
                    ones_row[:].bitcast(F32R),
                    start=False, stop=True, skip_group_check=True,
                )
                out_sb = sb.tile([128, BSH], BF16, tag=f"os{m}", name=f"os{m}")
                if m == 0:
                    nc.scalar.copy(out=out_sb[:], in_=pgs[m][:, 0:BSH])
                    nc.sync.dma_start(out=out_d[0:128, :], in_=out_sb[:])
                else:
                    nc.vector.tensor_copy(out_sb[:], pgs[m][:, 0:BSH])
                    nc.scalar.dma_start(out=out_d[128:256, :], in_=out_sb[:])

            gemm_part(0, wcT, xT)
            gemm_part(0, wqT, x2T)
            gemm_part(1, wcT, xT)
            reduce_m(0)
            finish_m(0)
            gemm_part(1, wqT, x2T)
            reduce_m(1)
            finish_m(1)

    nc.compile()
    return nc


def get_nc():
    if "nc" not in _CACHE:
        _CACHE["nc"] = _build()
    return _CACHE["nc"]


def kernel(x, mu, log_var, log_pi):
    x = np.asarray(x, dtype=np.float32).astype(ml_dtypes.bfloat16)
    mu = np.ascontiguousarray(
        np.asarray(mu, dtype=np.float32).astype(ml_dtypes.bfloat16)
    )
    lv = np.ascontiguousarray(
        np.asarray(log_var, dtype=np.float32).astype(ml_dtypes.bfloat16)
    )
    lp = np.ascontiguousarray(
        np.asarray(log_pi, dtype=np.float32).reshape(1, C)
    )
    nc = get_nc()
    in_maps = [
        {"x": np.ascontiguousarray(x[c * BSH:(c + 1) * BSH]),
         "mu": mu, "lv": lv, "lp": lp}
        for c in range(NCORES)
    ]
    res = run_bass_kernel_spmd(nc, in_maps, list(range(NCORES)))
    out = np.empty((B, C), dtype=np.float32)
    for c in range(NCORES):
        out[c * BSH:(c + 1) * BSH, :] = res.results[c]["out"].T.astype(np.float32)
    return out


# revision 15
# speedup vs baseline: 2.2687x; 1.0174x over previous
"""GaussianNB log-posterior kernel for 8 Trainium2 NeuronCores.

out[b, c] = log_pi[c] - 0.5 * sum_f(log2pi + log_var[c,f] + (x[b,f]-mu[c,f])^2 / var[c,f])

Strategy v5: data-parallel over batch (B=2048 -> 256 rows/core), mu/log_var
replicated; host casts x/mu/log_var to bf16 and reads back bf16 output
(rel err ~2e-3 vs the 2e-2 gate).

Per core:
  - 6 per-m-tile bf16 DMA loads over sync/scalar queues (x first).
  - PE warmup matmuls (HAM ungate), then interleaved PE transposes
    (x.b0, lv.m0, mu.m0, x.b1, lv.m1, mu.m1) so the m0 weight chain
    starts earliest.
  - Fused copybacks: invT=exp(-lvT), x2T=xT^2 via ACT straight from
    transpose PSUM; xT via DVE copy; mu/lv PSUM read directly by DVE:
    wcT=mu_ps*invT, wqT=-0.5*invT, m2iT=mu_ps*wcT, sT=lv_ps+m2iT.
  - PE GEMM outT[c,b] = sum_k wcT*xT + wqT*x2T into pg_m[:, 0:256];
    per-m ones-stationary row-reduce of sT (+ a K=1 matmul folding
    -2*log_pi) lands in the same PSUM bank at pg_m[0:1, 256:384];
    const_row = -0.5*(row) - 0.5*F*log2pi via one fused tensor_scalar,
    then folded into the GEMM psum as a trailing rank-1 f32r matmul.
Output per core is (C, 256) bf16 transposed slice; host reassembles.
"""
import sys

sys.path.insert(0, "/opt/trn_rl_repo")
import numpy as np
import ml_dtypes
import concourse.bacc as bacc
import concourse.mybir as mybir
from concourse.tile import TileContext
from concourse.bass_utils import run_bass_kernel_spmd
from concourse.masks import make_identity

B, C, F = 2048, 256, 1024
NCORES = 8
BSH = B // NCORES  # 256
KT = F // 128      # 8 k-tiles
LOG_2PI = float(np.log(2.0 * np.pi))
F32 = mybir.dt.float32
F32R = mybir.dt.float32r
BF16 = mybir.dt.bfloat16
AX = mybir.AxisListType.X
OP = mybir.AluOpType
AF = mybir.ActivationFunctionType

N_WARMUP = 10

_CACHE = {}


def _build():
    nc = bacc.Bacc("TRN2", target_bir_lowering=False, debug=False, num_devices=NCORES)
    x_d = nc.dram_tensor("x", [BSH, F], BF16, kind="ExternalInput").ap()
    mu_d = nc.dram_tensor("mu", [C, F], BF16, kind="ExternalInput").ap()
    lv_d = nc.dram_tensor("lv", [C, F], BF16, kind="ExternalInput").ap()
    lp_d = nc.dram_tensor("lp", [1, C], F32, kind="ExternalInput").ap()
    out_d = nc.dram_tensor("out", [C, BSH], BF16, kind="ExternalOutput").ap()

    with TileContext(nc) as tc:
        with (
            tc.tile_pool(name="sb", bufs=1) as sb,
            tc.tile_pool(name="tp", bufs=1, space="PSUM") as tp,
            tc.tile_pool(name="po", bufs=1, space="PSUM") as po,
        ):
            # ---------- per-m-tile DMA in (bf16), x first ----------
            lv_nat = sb.tile([128, 2, F], BF16, tag="lvn")
            mu_nat = sb.tile([128, 2, F], BF16, tag="mun")
            x_nat = sb.tile([128, 2, F], BF16, tag="xn")
            lp = sb.tile([1, C], F32, tag="lp")
            xv = x_d.rearrange("(m p) f -> p m f", p=128)
            lvv = lv_d.rearrange("(m p) f -> p m f", p=128)
            muv = mu_d.rearrange("(m p) f -> p m f", p=128)
            nc.scalar.dma_start(out=lp[:], in_=lp_d[:, :])
            nc.sync.dma_start(out=x_nat[:, 0], in_=xv[:, 0])
            nc.scalar.dma_start(out=mu_nat[:, 0], in_=muv[:, 0])
            nc.sync.dma_start(out=x_nat[:, 1], in_=xv[:, 1])
            nc.scalar.dma_start(out=mu_nat[:, 1], in_=muv[:, 1])
            nc.sync.dma_start(out=lv_nat[:, 0], in_=lvv[:, 0])
            nc.sync.dma_start(out=lv_nat[:, 1], in_=lvv[:, 1])

            # ---------- constants ----------
            ident = sb.tile([128, 128], F32, tag="id")
            make_identity(nc, ident[:])
            identb = sb.tile([128, 128], BF16, tag="idb")
            nc.gpsimd.tensor_copy(identb[:], ident[:])
            ones_big = sb.tile([128, 256], BF16, tag="oneb")
            nc.gpsimd.memset(ones_big[:], 1.0)
            ones_col = sb.tile([128, 1], BF16, tag="onec")
            nc.gpsimd.memset(ones_col[:], 1.0)
            ones_row = sb.tile([1, BSH], F32, tag="oner")
            nc.gpsimd.memset(ones_row[:], 1.0)

            # ---------- PSUM layout ----------
            # pg_m bank: [:, 0:256] GEMM out (c-tile m); [0:1, 256:384] s-row
            pg0 = po.tile([128, 512], F32, tag="pg0")
            pg1 = po.tile([128, 512], F32, tag="pg1")
            pgs = [pg0, pg1]

            # ---------- PE warmup (HAM ungate) during DMA window ----------
            for _ in range(N_WARMUP):
                nc.tensor.matmul(
                    pg0[:, 0:BSH], ones_big[:, 0:128], ones_big[:],
                    start=True, stop=True,
                )

            # ---------- PE transposes into PSUM quads ----------
            xT = sb.tile([128, KT, BSH], BF16, tag="xT")
            x2T = sb.tile([128, KT, BSH], BF16, tag="x2T")
            invT = sb.tile([128, KT, C], BF16, tag="invT")

            def transpose_m(nat, m, tag):
                p = tp.tile([128, KT * 128], BF16, tag=tag, name=tag)
                for k in range(KT):
                    nc.tensor.transpose(
                        p[:, k * 128:(k + 1) * 128],
                        nat[:, m, k * 128:(k + 1) * 128],
                        identb[:],
                    )
                return p[:].rearrange("p (k c) -> p k c", k=KT)

            lv_ps, mu_ps = [None, None], [None, None]

            def fillers(n):
                # keep the PE busy through DMA-wait gaps so HAM stays ungated
                for _ in range(n):
                    nc.tensor.matmul(
                        pg1[:, 0:BSH], ones_big[:, 0:128], ones_big[:],
                        start=True, stop=True,
                    )

            def x_round(b):
                ps = transpose_m(x_nat, b, f"xp{b}")
                sl = slice(b * 128, (b + 1) * 128)
                nc.vector.tensor_copy(xT[:, :, sl], ps)
                # x2T = (-0.5*x)*x in one fused op (mixed SBUF*PSUM srcs);
                # the -0.5 fold lets the quad GEMM consume invT directly
                nc.vector.scalar_tensor_tensor(
                    x2T[:, :, sl], ps, -0.5, xT[:, :, sl],
                    OP.mult, OP.mult,
                )

            def w_round(m):
                lv_ps[m] = transpose_m(lv_nat, m, f"lvp{m}")
                sl = slice(m * 128, (m + 1) * 128)
                nc.scalar.activation(invT[:, :, sl], lv_ps[m], AF.Exp, scale=-1.0)
                mu_ps[m] = transpose_m(mu_nat, m, f"mup{m}")

            x_round(0)
            fillers(3)
            x_round(1)
            fillers(3)
            w_round(0)
            fillers(3)
            w_round(1)
            fillers(2)

            # ---------- f-major elementwise prep (DVE, psum-direct) ----------
            wcT = sb.tile([128, KT, C], BF16, tag="wcT")
            m2iT = sb.tile([128, KT, C], BF16, tag="m2iT")
            sT = sb.tile([128, KT, C], BF16, tag="sT")
            for m in range(2):
                sl = slice(m * 128, (m + 1) * 128)
                nc.vector.tensor_mul(wcT[:, :, sl], mu_ps[m], invT[:, :, sl])
                nc.vector.tensor_mul(m2iT[:, :, sl], mu_ps[m], wcT[:, :, sl])
                nc.vector.tensor_add(sT[:, :, sl], lv_ps[m], m2iT[:, :, sl])
            lpm2 = sb.tile([1, C], BF16, tag="lpm2")
            nc.vector.tensor_scalar_mul(lpm2[:], lp[:], -2.0)

            # ---------- GEMM + per-m reduce + const + epilogue ----------
            const_row = sb.tile([1, C], F32R, tag="crow")

            def gemm_part(m, T, A, start=False):
                sl = slice(m * 128, (m + 1) * 128)
                for k in range(KT):
                    nc.tensor.matmul(
                        pgs[m][:, 0:BSH], T[:, k, sl], A[:, k, :],
                        start=(start and k == 0), stop=False,
                    )

            def reduce_m(m):
                sl = slice(m * 128, (m + 1) * 128)
                row = pgs[m][0:1, BSH:BSH + 128]
                # start=False everywhere: the GEMM's start=True already
                # cleared this bank (incl. the row region), so the first
                # write overwrites via has_written=0
                for k in range(KT):
                    nc.tensor.matmul(
                        row, ones_col[:], sT[:, k, sl],
                        start=False, stop=False, skip_group_check=True,
                    )
                nc.tensor.matmul(
                    row, ones_col[0:1, :], lpm2[:, sl],
                    start=False, stop=True, skip_group_check=True,
                )
                # const = -0.5*row - 0.5*F*log2pi  (one fused op from PSUM)
                nc.scalar.activation(
                    const_row[:, sl], row, AF.Copy,
                    bias=-0.5 * F * LOG_2PI, scale=-0.5,
                )

            def finish_m(m):
                sl = slice(m * 128, (m + 1) * 128)
                nc.tensor.matmul(
                    pgs[m][:, 0:BSH],
                    const_row[:, sl],
                    ones_row[:].bitcast(F32R),
                    start=False, stop=True, skip_group_check=True,
                )
                out_sb = sb.tile([128, BSH], BF16, tag=f"os{m}", name=f"os{m}")
                if m == 0:
                    nc.scalar.copy(out=out_sb[:], in_=pgs[m][:, 0:BSH])
                    nc.sync.dma_start(out=out_d[0:128, :], in_=out_sb[:])
                else:
                    nc.scalar.copy(out=out_sb[:], in_=pgs[m][:, 0:BSH])
                    nc.scalar.dma_start(out=out_d[128:256, :], in_=out_sb[:])

            gemm_part(0, invT, x2T, start=True)
            gemm_part(0, wcT, xT)
            gemm_part(1, invT, x2T, start=True)
            gemm_part(1, wcT, xT)
            reduce_m(0)
            reduce_m(1)
            finish_m(0)
            finish_m(1)

    nc.compile()
    return nc


def get_nc():
    if "nc" not in _CACHE:
        _CACHE["nc"] = _build()
    return _CACHE["nc"]


def kernel(x, mu, log_var, log_pi):
    x = np.asarray(x, dtype=np.float32).astype(ml_dtypes.bfloat16)
    mu = np.ascontiguousarray(
        np.asarray(mu, dtype=np.float32).astype(ml_dtypes.bfloat16)
    )
    lv = np.ascontiguousarray(
        np.asarray(log_var, dtype=np.float32).astype(ml_dtypes.bfloat16)
    )
    lp = np.ascontiguousarray(
        np.asarray(log_pi, dtype=np.float32).reshape(1, C)
    )
    nc = get_nc()
    in_maps = [
        {"x": np.ascontiguousarray(x[c * BSH:(c + 1) * BSH]),
         "mu": mu, "lv": lv, "lp": lp}
        for c in range(NCORES)
    ]
    res = run_bass_kernel_spmd(nc, in_maps, list(range(NCORES)))
    out = np.empty((B, C), dtype=np.float32)
    for c in range(NCORES):
        out[c * BSH:(c + 1) * BSH, :] = res.results[c]["out"].T.astype(np.float32)
    return out
